# revision 2
# baseline (speedup 1.0000x reference)
"""FAConv GNN message-passing kernel for 8 TRN2 NeuronCores.

Sharding strategy (per the node/edge-partition hint):
- Nodes sharded across 8 cores (12500 each = 98 blocks of 128).
- Edges partitioned by destination core/block. Each core's shard of the
  edge list is distributed together with the source- and destination-node
  feature rows those edges touch (the halo exchange is resolved at input
  distribution time: the per-edge x_src / x_dst feature rows are laid
  out in edge order on the host, which only does indexing/layout).
- att_l/att_r/W/b are tiny and folded into replicated constants.

Device pipeline per core (all FLOPs on device):
- xsg [128, C, 128] bf16 holds 128-edge columns of x_src rows (edge lane
  on partitions); xdg [128, C, 128] holds x_dst transposed per column
  (feature on partitions) so ar = x_dst . att_r is a per-column PE matvec.
- al per edge: batched multiply by att_l plus a binary-tree reduction
  over the 128-wide free axis (DVE).
- coeff = edge_weight * tanh(al + ar), batched over column groups.
- Scatter-add per destination block b: PSUM accumulates
  0.1*x0 (via a 0.1*I matmul) plus, per column, XS^T @ Ssc where
  Ssc[e, i] = (i == dst_e) * coeff_e built by one gpsimd local_scatter.
- Postlude per block: relu, output Linear (W^T matmul + bias), y in bf16.

Pipelining: xsg/xdg/x0t are streamed per group with deep tile pools so
the 16 DMA engines stay saturated; the dst-half tiles free early (their
only reader, the ar matmuls, runs first).
"""

import numpy as np
import ml_dtypes

import concourse.bacc as bacc
import concourse.mybir as mybir
import concourse.tile as tile
from concourse.library_config import local_scatter as local_scatter_lib

BF = ml_dtypes.bfloat16
F32 = mybir.dt.float32
F16 = mybir.dt.float16
BF16 = mybir.dt.bfloat16
I16 = mybir.dt.int16

EPS = 0.1
D = 128
N_CORES = 8
P = 128
GROUP_BLOCKS = 4
WIN = 14  # columns per local_scatter window (num_elems = WIN*128 <= 2046)


def _ceil(a, b):
    return (a + b - 1) // b


def make_plan(edge_index, n_nodes, n_cores):
    """Index-only preprocessing: partition edges by destination core and
    block, lay them out in 128-edge columns (shared column layout across
    cores, padded to the per-block max)."""
    src = np.asarray(edge_index[0], np.int64)
    dst = np.asarray(edge_index[1], np.int64)
    n_loc = n_nodes // n_cores
    n_blk = _ceil(n_loc, P)

    per_core_edges = []
    blk_counts = np.zeros((n_cores, n_blk), np.int64)
    for c in range(n_cores):
        m = (dst >= c * n_loc) & (dst < (c + 1) * n_loc)
        s = src[m]
        d_loc = dst[m] - c * n_loc
        w_pos = np.nonzero(m)[0]
        blk = d_loc >> 7
        order = np.argsort(blk, kind="stable")
        per_core_edges.append((s[order], d_loc[order], w_pos[order], blk[order]))
        np.add.at(blk_counts[c], blk[order], 1)

    cols_per_blk = np.maximum(1, -(-blk_counts.max(axis=0) // P))  # [n_blk]
    col_off = np.concatenate([[0], np.cumsum(cols_per_blk)])
    n_cols = int(col_off[-1])

    per_core = []
    for c in range(n_cores):
        s, d_loc, w_pos, blk = per_core_edges[c]
        # flat slot of edge k within block b: (col_off[b] + k//128)*128 + k%128
        k_in_blk = np.arange(len(s)) - np.concatenate(
            [[0], np.cumsum(blk_counts[c])]
        )[blk]
        slot = (col_off[blk] + (k_in_blk >> 7)) * P + (k_in_blk & 127)
        srcm = np.zeros(n_cols * P, np.int64)
        dstg = np.zeros(n_cols * P, np.int64)
        dstl = np.zeros(n_cols * P, np.float32)
        wsel = np.zeros(n_cols * P, np.int64)
        wval = np.zeros(n_cols * P, bool)
        srcm[slot] = s
        dstg[slot] = d_loc + c * n_loc
        dstl[slot] = d_loc & 127
        wsel[slot] = w_pos
        wval[slot] = True
        # [n_cols*P] flat (col-major slots) -> [P, n_cols]
        per_core.append(
            {
                "srcm": srcm.reshape(n_cols, P).T,
                "dstg": dstg.reshape(n_cols, P).T,
                "dstl": np.ascontiguousarray(dstl.reshape(n_cols, P).T),
                "wsel": wsel.reshape(n_cols, P).T,
                "wval": wval.reshape(n_cols, P).T,
            }
        )

    # block groups and scatter windows (group-local, WIN columns each).
    # Each group gets its own dst16 region at an EVEN column base (the
    # gpsimd local_scatter ucode needs 4-byte-aligned operand offsets),
    # with a sentinel (-1) column so odd-width windows can pad num_idxs
    # to even without reading a neighbor's column.
    groups = []
    for g0 in range(0, n_blk, GROUP_BLOCKS):
        groups.append(list(range(g0, min(g0 + GROUP_BLOCKS, n_blk))))
    win_col = np.zeros(n_cols, np.int64)  # column offset within its window
    dbase = []  # even dst16 base per group
    pos = 0
    for blks in groups:
        c0, c1 = int(col_off[blks[0]]), int(col_off[blks[-1] + 1])
        win_col[c0:c1] = (np.arange(c1 - c0)) % WIN
        dbase.append(pos)
        pos += (c1 - c0) + 1
        pos += pos & 1

    n_cols_pad = pos
    for pc in per_core:
        enc = np.where(
            pc["wval"], (win_col[None, :] * P + pc["dstl"]).astype(np.int64), -1
        ).astype(np.int16)
        dst16 = np.full((P, n_cols_pad), -1, np.int16)
        for gi, blks in enumerate(groups):
            c0, c1 = int(col_off[blks[0]]), int(col_off[blks[-1] + 1])
            dst16[:, dbase[gi] : dbase[gi] + (c1 - c0)] = enc[:, c0:c1]
        pc["dst16"] = dst16

    plan = {
        "n_nodes": n_nodes,
        "n_cores": n_cores,
        "n_loc": n_loc,
        "n_blk": n_blk,
        "npad": n_blk * P,
        "n_cols": n_cols,
        "cols_per_blk": cols_per_blk,
        "col_off": col_off,
        "groups": groups,
        "n_cols_pad": n_cols_pad,
        "dbase": dbase,
    }
    return plan, per_core


def build_nc(plan):
    n_blk = plan["n_blk"]
    n_cols = plan["n_cols"]
    npad = plan["npad"]
    cols_per_blk = plan["cols_per_blk"]
    col_off = plan["col_off"]

    nc = bacc.Bacc(None, target_bir_lowering=False)

    xsg_d = nc.dram_tensor("xsg", [P, n_cols, D], BF16, kind="ExternalInput")
    xdg_d = nc.dram_tensor("xdg", [P, n_cols, D], BF16, kind="ExternalInput")
    dst_d = nc.dram_tensor("dst16", [P, plan["n_cols_pad"]], I16, kind="ExternalInput")
    w_d = nc.dram_tensor("wf", [P, n_cols], F16, kind="ExternalInput")
    x0t_d = nc.dram_tensor("x0t", [P, npad], BF16, kind="ExternalInput")
    attl_d = nc.dram_tensor("attl_rep", [P, D], BF16, kind="ExternalInput")
    attr_d = nc.dram_tensor("attr_col", [P, 1], BF16, kind="ExternalInput")
    ideps_d = nc.dram_tensor("ideps", [P, P], BF16, kind="ExternalInput")
    wdo_d = nc.dram_tensor("w_do", [P, P], BF16, kind="ExternalInput")
    bcol_d = nc.dram_tensor("b_col", [P, 1], F32, kind="ExternalInput")
    yt_d = nc.dram_tensor("yt", [P, npad], BF16, kind="ExternalOutput")

    groups = plan["groups"]
    cg_max = max(
        int(col_off[blks[-1] + 1] - col_off[blks[0]]) for blks in groups
    )
    gb_max = max(len(blks) for blks in groups)

    nc.gpsimd.load_library(local_scatter_lib)

    with tile.TileContext(nc) as tc:
        with (
            tc.tile_pool(name="const", bufs=1) as constp,
            tc.tile_pool(name="xsg", bufs=8) as xsgp,
            tc.tile_pool(name="xdg", bufs=6) as xdgp,
            tc.tile_pool(name="x0c", bufs=4) as x0p,
            tc.tile_pool(name="tree", bufs=2) as treep,
            tc.tile_pool(name="col", bufs=4) as colp,
            tc.tile_pool(name="ssc", bufs=6) as sscp,
            tc.tile_pool(name="out", bufs=3) as outp,
            tc.tile_pool(name="psA", bufs=2, space="PSUM") as psA,
            tc.tile_pool(name="psB", bufs=2, space="PSUM") as psB,
            tc.tile_pool(name="psC", bufs=2, space="PSUM") as psC,
        ):
            dst_sb = constp.tile([P, plan["n_cols_pad"]], I16)
            nc.sync.dma_start(out=dst_sb[:], in_=dst_d[:])
            w_sb = constp.tile([P, n_cols], F16)
            nc.sync.dma_start(out=w_sb[:], in_=w_d[:])
            attl_sb = constp.tile([P, D], BF16)
            nc.sync.dma_start(out=attl_sb[:], in_=attl_d[:])
            attr_sb = constp.tile([P, 1], BF16)
            nc.sync.dma_start(out=attr_sb[:], in_=attr_d[:])
            ideps_sb = constp.tile([P, P], BF16)
            nc.sync.dma_start(out=ideps_sb[:], in_=ideps_d[:])
            wdo_sb = constp.tile([P, P], BF16)
            nc.sync.dma_start(out=wdo_sb[:], in_=wdo_d[:])
            bcol_sb = constp.tile([P, 1], F32)
            nc.sync.dma_start(out=bcol_sb[:], in_=bcol_d[:])

            for gi, blocks in enumerate(groups):
                c0 = int(col_off[blocks[0]])
                c1 = int(col_off[blocks[-1] + 1])
                cg = c1 - c0
                gb = len(blocks)

                xdg = xdgp.tile([P, cg_max, D], BF16, tag="xdg")
                nc.sync.dma_start(out=xdg[:, :cg, :], in_=xdg_d[:, c0:c1, :])
                xsg = xsgp.tile([P, cg_max, D], BF16, tag="xsg")
                nc.sync.dma_start(out=xsg[:, :cg, :], in_=xsg_d[:, c0:c1, :])
                x0c = x0p.tile([P, gb_max * P], BF16, tag="x0c")
                nc.sync.dma_start(
                    out=x0c[:, : gb * P],
                    in_=x0t_d[:, blocks[0] * P : (blocks[-1] + 1) * P],
                )

                # ar[e] = sum_d xd[e,d]*att_r[d]: the dst half is stored
                # TRANSPOSED per column, so ar is a per-column PE matvec.
                # Emitted first: it is the only reader of xdg, so the dst
                # tile recycles early.
                ar_ps = psC.tile([P, cg_max], F32, space="PSUM", tag="ar")
                for j in range(cg):
                    nc.tensor.matmul(
                        out=ar_ps[:, j : j + 1],
                        lhsT=xdg[:, j, :],
                        rhs=attr_sb[:],
                        start=True,
                        stop=True,
                    )

                # al[e] = sum_d xs[e,d]*att_l[d]: multiply + binary-tree (DVE)
                prod = treep.tile([P, cg_max, D], F16, tag="prod")
                nc.vector.tensor_tensor(
                    out=prod[:, :cg, :],
                    in0=xsg[:, :cg, :],
                    in1=attl_sb[:].unsqueeze(1).to_broadcast([P, cg, D]),
                    op=mybir.AluOpType.mult,
                )
                width = D
                cur = prod
                while width >= 2:
                    nxt = treep.tile([P, cg_max, width // 2], F16, tag=f"t{width}")
                    nc.vector.tensor_tensor(
                        out=nxt[:, :cg, :],
                        in0=cur[:, :cg, : width // 2],
                        in1=cur[:, :cg, width // 2 : width],
                        op=mybir.AluOpType.add,
                    )
                    cur = nxt
                    width //= 2

                alr = colp.tile([P, cg_max], F16, tag="alr")
                nc.vector.tensor_tensor(
                    out=alr[:, :cg],
                    in0=cur[:, :cg, 0],
                    in1=ar_ps[:, :cg],
                    op=mybir.AluOpType.add,
                )
                th = colp.tile([P, cg_max], F16, tag="th")
                nc.scalar.activation(
                    out=th[:, :cg],
                    in_=alr[:, :cg],
                    func=mybir.ActivationFunctionType.Tanh,
                )
                co = colp.tile([P, cg_max + 1], BF16, tag="co")
                nc.vector.tensor_tensor(
                    out=co[:, :cg],
                    in0=th[:, :cg],
                    in1=w_sb[:, c0:c1],
                    op=mybir.AluOpType.mult,
                )

                # one-hot scatter columns for the group's windows (GpSimd)
                wins = []
                for wk in range(0, cg, WIN):
                    w0, w1 = wk, min(wk + WIN, cg)
                    nw = w1 - w0
                    nw_pad = nw + (nw & 1)
                    db = plan["dbase"][gi]
                    scat = sscp.tile([P, WIN * P], BF16, tag="scat")
                    nc.gpsimd.local_scatter(
                        out_ap=scat[:, : nw * P],
                        data_ap=co[:, w0 : w0 + nw_pad],
                        idxs_ap=dst_sb[:, db + w0 : db + w0 + nw_pad],
                        channels=P,
                        num_elems=nw * P,
                        num_idxs=nw_pad,
                    )
                    wins.append(scat)

                yg = outp.tile([P, gb_max * P], BF16, tag="yg")
                for bi, b in enumerate(blocks):
                    nb = int(cols_per_blk[b])
                    agg = psA.tile([P, P], F32, space="PSUM", tag="agg")
                    nc.tensor.matmul(
                        out=agg[:],
                        lhsT=ideps_sb[:],
                        rhs=x0c[:, bi * P : (bi + 1) * P],
                        start=True,
                        stop=False,
                    )
                    for j in range(nb):
                        c = int(col_off[b]) + j
                        wk, wc = divmod(c - c0, WIN)
                        nc.tensor.matmul(
                            out=agg[:],
                            lhsT=xsg[:, c - c0, :],
                            rhs=wins[wk][:, wc * P : (wc + 1) * P],
                            start=False,
                            stop=(j == nb - 1),
                        )
                    reluT = outp.tile([P, P], BF16, tag="reluT")
                    nc.scalar.activation(
                        out=reluT[:],
                        in_=agg[:],
                        func=mybir.ActivationFunctionType.Relu,
                    )
                    y_ps = psB.tile([P, P], F32, space="PSUM", tag="y")
                    nc.tensor.matmul(
                        out=y_ps[:],
                        lhsT=wdo_sb[:],
                        rhs=reluT[:],
                        start=True,
                        stop=True,
                    )
                    nc.scalar.activation(
                        out=yg[:, bi * P : (bi + 1) * P],
                        in_=y_ps[:],
                        func=mybir.ActivationFunctionType.Identity,
                        bias=bcol_sb[:],
                    )
                nc.sync.dma_start(
                    out=yt_d[:, blocks[0] * P : (blocks[-1] + 1) * P],
                    in_=yg[:, : gb * P],
                )

    nc.finalize()
    return nc


def _prep_inputs(plan, per_core, x, x_0, edge_weight, att_l, att_r, W, b):
    n_loc, n_blk, npad = plan["n_loc"], plan["n_blk"], plan["npad"]
    n_cores = plan["n_cores"]

    xb16 = np.ascontiguousarray(np.asarray(x, np.float32)).astype(BF)
    xb_u16 = xb16.view(np.uint16)
    attl_rep = np.tile(np.asarray(att_l, np.float32)[None, :], (P, 1)).astype(BF)
    attr_col = np.asarray(att_r, np.float32)[:, None].astype(BF)
    ideps = (EPS * np.eye(P, dtype=np.float64)).astype(BF)
    w_do = np.ascontiguousarray(np.asarray(W, np.float32).T).astype(BF)
    b_col = np.asarray(b, np.float32)[:, None]
    ew = np.asarray(edge_weight, np.float32)

    in_maps = []
    for c in range(n_cores):
        pc = per_core[c]
        n_cols = plan["n_cols"]
        xsg = np.ascontiguousarray(xb_u16[pc["srcm"]])
        # dst stored transposed per column: xdg[d, c, e] = x[dst[e,c], d]
        xdg = np.ascontiguousarray(xb_u16[pc["dstg"]].transpose(2, 1, 0))
        wf = np.where(pc["wval"], ew[pc["wsel"]], 0.0).astype(np.float16)
        x0_loc = np.zeros((npad, D), np.float32)
        x0_loc[:n_loc] = np.asarray(x_0[c * n_loc : (c + 1) * n_loc], np.float32)
        x0t = np.ascontiguousarray(x0_loc.T).astype(BF)
        in_maps.append(
            {
                "xsg": xsg.view(BF),
                "xdg": xdg.view(BF),
                "dst16": pc["dst16"],
                "wf": wf,
                "x0t": x0t,
                "attl_rep": attl_rep,
                "attr_col": attr_col,
                "ideps": ideps,
                "w_do": w_do,
                "b_col": b_col,
            }
        )
    return in_maps


def kernel(x, x_0, edge_weight, att_l, att_r, W, b, edge_index):
    from concourse.bass_utils import run_bass_kernel_spmd

    n_nodes = x.shape[0]
    plan, per_core = make_plan(edge_index, n_nodes, N_CORES)
    nc = build_nc(plan)
    in_maps = _prep_inputs(plan, per_core, x, x_0, edge_weight, att_l, att_r, W, b)
    res = run_bass_kernel_spmd(nc, in_maps, core_ids=list(range(N_CORES)))
    n_loc = plan["n_loc"]
    out = np.empty((n_nodes, P), np.float32)
    for c in range(N_CORES):
        out[c * n_loc : (c + 1) * n_loc] = (
            res.results[c]["yt"].T[:n_loc].astype(np.float32)
        )
    return out


# revision 11
# speedup vs baseline: 1.0639x; 1.0639x over previous
"""FAConv GNN message-passing kernel for 8 TRN2 NeuronCores.

Sharding strategy (per the node/edge-partition hint):
- Nodes sharded across 8 cores (12500 each = 98 blocks of 128).
- Edges partitioned by destination core/block. Each core's shard of the
  edge list is distributed together with the source- and destination-node
  feature rows those edges touch (the halo exchange is resolved at input
  distribution time: the per-edge x_src / x_dst feature rows are laid
  out in edge order on the host, which only does indexing/layout).
- att_l/att_r/W/b are tiny and folded into replicated constants.

Device pipeline per core (all FLOPs on device):
- xsg [128, C, 128] bf16 holds 128-edge columns of x_src rows (edge lane
  on partitions); xdg [128, C, 128] holds x_dst transposed per column
  (feature on partitions) so ar = x_dst . att_r is a per-column PE matvec.
- al per edge: batched multiply by att_l plus a binary-tree reduction
  over the 128-wide free axis (DVE).
- coeff = edge_weight * tanh(al + ar), batched over column groups.
- Scatter-add per destination block b: PSUM accumulates
  0.1*x0 (via a 0.1*I matmul) plus, per column, XS^T @ Ssc where
  Ssc[e, i] = (i == dst_e) * coeff_e built by one gpsimd local_scatter.
- Postlude per block: relu, output Linear (W^T matmul + bias), y in bf16.

Pipelining: xsg/xdg/x0t are streamed per group with deep tile pools so
the 16 DMA engines stay saturated; the dst-half tiles free early (their
only reader, the ar matmuls, runs first).
"""

import numpy as np
import ml_dtypes

import concourse.bacc as bacc
import concourse.mybir as mybir
import concourse.tile as tile
from concourse.library_config import local_scatter as local_scatter_lib

BF = ml_dtypes.bfloat16
F32 = mybir.dt.float32
F16 = mybir.dt.float16
BF16 = mybir.dt.bfloat16
I16 = mybir.dt.int16

EPS = 0.1
D = 128
N_CORES = 8
P = 128
GROUP_BLOCKS = 4
WIN = 14  # columns per local_scatter window (num_elems = WIN*128 <= 2046)


def _ceil(a, b):
    return (a + b - 1) // b


def make_plan(edge_index, n_nodes, n_cores):
    """Index-only preprocessing: partition edges by destination core and
    block, lay them out in 128-edge columns (shared column layout across
    cores, padded to the per-block max)."""
    src = np.asarray(edge_index[0], np.int64)
    dst = np.asarray(edge_index[1], np.int64)
    n_loc = n_nodes // n_cores
    n_blk = _ceil(n_loc, P)

    per_core_edges = []
    blk_counts = np.zeros((n_cores, n_blk), np.int64)
    for c in range(n_cores):
        m = (dst >= c * n_loc) & (dst < (c + 1) * n_loc)
        s = src[m]
        d_loc = dst[m] - c * n_loc
        w_pos = np.nonzero(m)[0]
        blk = d_loc >> 7
        order = np.argsort(blk, kind="stable")
        per_core_edges.append((s[order], d_loc[order], w_pos[order], blk[order]))
        np.add.at(blk_counts[c], blk[order], 1)

    cols_per_blk = np.maximum(1, -(-blk_counts.max(axis=0) // P))  # [n_blk]
    col_off = np.concatenate([[0], np.cumsum(cols_per_blk)])
    n_cols = int(col_off[-1])

    per_core = []
    for c in range(n_cores):
        s, d_loc, w_pos, blk = per_core_edges[c]
        # flat slot of edge k within block b: (col_off[b] + k//128)*128 + k%128
        k_in_blk = np.arange(len(s)) - np.concatenate(
            [[0], np.cumsum(blk_counts[c])]
        )[blk]
        slot = (col_off[blk] + (k_in_blk >> 7)) * P + (k_in_blk & 127)
        srcm = np.zeros(n_cols * P, np.int64)
        dstg = np.zeros(n_cols * P, np.int64)
        dstl = np.zeros(n_cols * P, np.float32)
        wsel = np.zeros(n_cols * P, np.int64)
        wval = np.zeros(n_cols * P, bool)
        srcm[slot] = s
        dstg[slot] = d_loc + c * n_loc
        dstl[slot] = d_loc & 127
        wsel[slot] = w_pos
        wval[slot] = True
        # [n_cols*P] flat (col-major slots) -> [P, n_cols]
        per_core.append(
            {
                "srcm": srcm.reshape(n_cols, P).T,
                "dstg": dstg.reshape(n_cols, P).T,
                "dstl": np.ascontiguousarray(dstl.reshape(n_cols, P).T),
                "wsel": wsel.reshape(n_cols, P).T,
                "wval": wval.reshape(n_cols, P).T,
            }
        )

    # block groups and scatter windows (group-local, WIN columns each).
    # Each group gets its own dst16 region at an EVEN column base (the
    # gpsimd local_scatter ucode needs 4-byte-aligned operand offsets),
    # with a sentinel (-1) column so odd-width windows can pad num_idxs
    # to even without reading a neighbor's column.
    groups = []
    for g0 in range(0, n_blk, GROUP_BLOCKS):
        groups.append(list(range(g0, min(g0 + GROUP_BLOCKS, n_blk))))
    win_col = np.zeros(n_cols, np.int64)  # column offset within its window
    dbase = []  # even dst16 base per group
    pos = 0
    for blks in groups:
        c0, c1 = int(col_off[blks[0]]), int(col_off[blks[-1] + 1])
        win_col[c0:c1] = (np.arange(c1 - c0)) % WIN
        dbase.append(pos)
        pos += (c1 - c0) + 1
        pos += pos & 1

    n_cols_pad = pos
    for pc in per_core:
        enc = np.where(
            pc["wval"], (win_col[None, :] * P + pc["dstl"]).astype(np.int64), -1
        ).astype(np.int16)
        dst16 = np.full((P, n_cols_pad), -1, np.int16)
        for gi, blks in enumerate(groups):
            c0, c1 = int(col_off[blks[0]]), int(col_off[blks[-1] + 1])
            dst16[:, dbase[gi] : dbase[gi] + (c1 - c0)] = enc[:, c0:c1]
        pc["dst16"] = dst16

    plan = {
        "n_nodes": n_nodes,
        "n_cores": n_cores,
        "n_loc": n_loc,
        "n_blk": n_blk,
        "npad": n_blk * P,
        "n_cols": n_cols,
        "cols_per_blk": cols_per_blk,
        "col_off": col_off,
        "groups": groups,
        "n_cols_pad": n_cols_pad,
        "dbase": dbase,
    }
    return plan, per_core


def build_nc(plan):
    n_blk = plan["n_blk"]
    n_cols = plan["n_cols"]
    npad = plan["npad"]
    cols_per_blk = plan["cols_per_blk"]
    col_off = plan["col_off"]

    nc = bacc.Bacc(None, target_bir_lowering=False)

    xgp_d = nc.dram_tensor("xgp", [P, n_cols, 2 * D], BF16, kind="ExternalInput")
    dst_d = nc.dram_tensor("dst16", [P, plan["n_cols_pad"]], I16, kind="ExternalInput")
    w_d = nc.dram_tensor("wf", [P, n_cols], F16, kind="ExternalInput")
    x0t_d = nc.dram_tensor("x0t", [P, npad], BF16, kind="ExternalInput")
    attl_d = nc.dram_tensor("attl_rep", [P, D], BF16, kind="ExternalInput")
    attr_d = nc.dram_tensor("attr_col", [P, 1], BF16, kind="ExternalInput")
    ideps_d = nc.dram_tensor("ideps", [P, P], BF16, kind="ExternalInput")
    wdo_d = nc.dram_tensor("w_do", [P, P], BF16, kind="ExternalInput")
    bcol_d = nc.dram_tensor("b_col", [P, 1], F32, kind="ExternalInput")
    yt_d = nc.dram_tensor("yt", [P, npad], BF16, kind="ExternalOutput")

    groups = plan["groups"]
    cg_max = max(
        int(col_off[blks[-1] + 1] - col_off[blks[0]]) for blks in groups
    )
    gb_max = max(len(blks) for blks in groups)

    nc.gpsimd.load_library(local_scatter_lib)

    with tile.TileContext(nc) as tc:
        with (
            tc.tile_pool(name="const", bufs=1) as constp,
            tc.tile_pool(name="xgp", bufs=6) as xgpp,
            tc.tile_pool(name="tree", bufs=2) as treep,
            tc.tile_pool(name="col", bufs=4) as colp,
            tc.tile_pool(name="ssc", bufs=6) as sscp,
            tc.tile_pool(name="out", bufs=3) as outp,
            tc.tile_pool(name="psA", bufs=2, space="PSUM") as psA,
            tc.tile_pool(name="psB", bufs=2, space="PSUM") as psB,
            tc.tile_pool(name="psC", bufs=2, space="PSUM") as psC,
        ):
            dst_sb = constp.tile([P, plan["n_cols_pad"]], I16)
            nc.sync.dma_start(out=dst_sb[:], in_=dst_d[:])
            w_sb = constp.tile([P, n_cols], F16)
            nc.sync.dma_start(out=w_sb[:], in_=w_d[:])
            attl_sb = constp.tile([P, D], BF16)
            nc.sync.dma_start(out=attl_sb[:], in_=attl_d[:])
            attr_sb = constp.tile([P, 1], BF16)
            nc.sync.dma_start(out=attr_sb[:], in_=attr_d[:])
            ideps_sb = constp.tile([P, P], BF16)
            nc.sync.dma_start(out=ideps_sb[:], in_=ideps_d[:])
            wdo_sb = constp.tile([P, P], BF16)
            nc.sync.dma_start(out=wdo_sb[:], in_=wdo_d[:])
            bcol_sb = constp.tile([P, 1], F32)
            nc.sync.dma_start(out=bcol_sb[:], in_=bcol_d[:])
            x0_sb = constp.tile([P, npad], BF16)
            nc.sync.dma_start(out=x0_sb[:], in_=x0t_d[:])

            for gi, blocks in enumerate(groups):
                c0 = int(col_off[blocks[0]])
                c1 = int(col_off[blocks[-1] + 1])
                cg = c1 - c0
                gb = len(blocks)

                xgp = xgpp.tile([P, cg_max, 2 * D], BF16, tag="xgp")
                nc.sync.dma_start(out=xgp[:, :cg, :], in_=xgp_d[:, c0:c1, :])

                # ar[e] = sum_d xd[e,d]*att_r[d]: the dst half is stored
                # TRANSPOSED per column, so ar is a per-column PE matvec.
                # Emitted first: it is the only reader of xdg, so the dst
                # tile recycles early.
                ar_ps = psC.tile([P, cg_max], F32, space="PSUM", tag="ar")
                for j in range(cg):
                    nc.tensor.matmul(
                        out=ar_ps[:, j : j + 1],
                        lhsT=xgp[:, j, D:],
                        rhs=attr_sb[:],
                        start=True,
                        stop=True,
                    )

                # al[e] = sum_d xs[e,d]*att_l[d]: multiply + binary-tree (DVE)
                prod = treep.tile([P, cg_max, D], F16, tag="prod")
                nc.vector.tensor_tensor(
                    out=prod[:, :cg, :],
                    in0=xgp[:, :cg, :D],
                    in1=attl_sb[:].unsqueeze(1).to_broadcast([P, cg, D]),
                    op=mybir.AluOpType.mult,
                )
                width = D
                cur = prod
                while width >= 2:
                    nxt = treep.tile([P, cg_max, width // 2], F16, tag=f"t{width}")
                    nc.vector.tensor_tensor(
                        out=nxt[:, :cg, :],
                        in0=cur[:, :cg, : width // 2],
                        in1=cur[:, :cg, width // 2 : width],
                        op=mybir.AluOpType.add,
                    )
                    cur = nxt
                    width //= 2

                alr = colp.tile([P, cg_max], F16, tag="alr")
                nc.vector.tensor_tensor(
                    out=alr[:, :cg],
                    in0=cur[:, :cg, 0],
                    in1=ar_ps[:, :cg],
                    op=mybir.AluOpType.add,
                )
                th = colp.tile([P, cg_max], F16, tag="th")
                nc.scalar.activation(
                    out=th[:, :cg],
                    in_=alr[:, :cg],
                    func=mybir.ActivationFunctionType.Tanh,
                )
                co = colp.tile([P, cg_max + 1], BF16, tag="co")
                nc.vector.tensor_tensor(
                    out=co[:, :cg],
                    in0=th[:, :cg],
                    in1=w_sb[:, c0:c1],
                    op=mybir.AluOpType.mult,
                )

                # one-hot scatter columns for the group's windows (GpSimd)
                wins = []
                for wk in range(0, cg, WIN):
                    w0, w1 = wk, min(wk + WIN, cg)
                    nw = w1 - w0
                    nw_pad = nw + (nw & 1)
                    db = plan["dbase"][gi]
                    scat = sscp.tile([P, WIN * P], BF16, tag="scat")
                    nc.gpsimd.local_scatter(
                        out_ap=scat[:, : nw * P],
                        data_ap=co[:, w0 : w0 + nw_pad],
                        idxs_ap=dst_sb[:, db + w0 : db + w0 + nw_pad],
                        channels=P,
                        num_elems=nw * P,
                        num_idxs=nw_pad,
                    )
                    wins.append(scat)

                yg = outp.tile([P, gb_max * P], BF16, tag="yg")
                for bi, b in enumerate(blocks):
                    nb = int(cols_per_blk[b])
                    agg = psA.tile([P, P], F32, space="PSUM", tag="agg")
                    nc.tensor.matmul(
                        out=agg[:],
                        lhsT=ideps_sb[:],
                        rhs=x0_sb[:, b * P : (b + 1) * P],
                        start=True,
                        stop=False,
                    )
                    for j in range(nb):
                        c = int(col_off[b]) + j
                        wk, wc = divmod(c - c0, WIN)
                        nc.tensor.matmul(
                            out=agg[:],
                            lhsT=xgp[:, c - c0, :D],
                            rhs=wins[wk][:, wc * P : (wc + 1) * P],
                            start=False,
                            stop=(j == nb - 1),
                        )
                    reluT = outp.tile([P, P], BF16, tag="reluT")
                    nc.scalar.activation(
                        out=reluT[:],
                        in_=agg[:],
                        func=mybir.ActivationFunctionType.Relu,
                    )
                    y_ps = psB.tile([P, P], F32, space="PSUM", tag="y")
                    nc.tensor.matmul(
                        out=y_ps[:],
                        lhsT=wdo_sb[:],
                        rhs=reluT[:],
                        start=True,
                        stop=True,
                    )
                    nc.scalar.activation(
                        out=yg[:, bi * P : (bi + 1) * P],
                        in_=y_ps[:],
                        func=mybir.ActivationFunctionType.Identity,
                        bias=bcol_sb[:],
                    )
                nc.sync.dma_start(
                    out=yt_d[:, blocks[0] * P : (blocks[-1] + 1) * P],
                    in_=yg[:, : gb * P],
                )

    nc.finalize()
    return nc


def _prep_inputs(plan, per_core, x, x_0, edge_weight, att_l, att_r, W, b):
    n_loc, n_blk, npad = plan["n_loc"], plan["n_blk"], plan["npad"]
    n_cores = plan["n_cores"]

    xb16 = np.ascontiguousarray(np.asarray(x, np.float32)).astype(BF)
    xb_u16 = xb16.view(np.uint16)
    attl_rep = np.tile(np.asarray(att_l, np.float32)[None, :], (P, 1)).astype(BF)
    attr_col = np.asarray(att_r, np.float32)[:, None].astype(BF)
    ideps = (EPS * np.eye(P, dtype=np.float64)).astype(BF)
    w_do = np.ascontiguousarray(np.asarray(W, np.float32).T).astype(BF)
    b_col = np.asarray(b, np.float32)[:, None]
    ew = np.asarray(edge_weight, np.float32)

    in_maps = []
    for c in range(n_cores):
        pc = per_core[c]
        n_cols = plan["n_cols"]
        xgp = np.empty((P, n_cols, 2 * D), np.uint16)
        xgp[:, :, :D] = xb_u16[pc["srcm"]]
        # dst half stored transposed per column: xgp[d, c, D+e] = x[dst[e,c], d]
        xgp[:, :, D:] = xb_u16[pc["dstg"]].transpose(2, 1, 0)
        wf = np.where(pc["wval"], ew[pc["wsel"]], 0.0).astype(np.float16)
        x0_loc = np.zeros((npad, D), np.float32)
        x0_loc[:n_loc] = np.asarray(x_0[c * n_loc : (c + 1) * n_loc], np.float32)
        x0t = np.ascontiguousarray(x0_loc.T).astype(BF)
        in_maps.append(
            {
                "xgp": xgp.view(BF),
                "dst16": pc["dst16"],
                "wf": wf,
                "x0t": x0t,
                "attl_rep": attl_rep,
                "attr_col": attr_col,
                "ideps": ideps,
                "w_do": w_do,
                "b_col": b_col,
            }
        )
    return in_maps


def kernel(x, x_0, edge_weight, att_l, att_r, W, b, edge_index):
    from concourse.bass_utils import run_bass_kernel_spmd

    n_nodes = x.shape[0]
    plan, per_core = make_plan(edge_index, n_nodes, N_CORES)
    nc = build_nc(plan)
    in_maps = _prep_inputs(plan, per_core, x, x_0, edge_weight, att_l, att_r, W, b)
    res = run_bass_kernel_spmd(nc, in_maps, core_ids=list(range(N_CORES)))
    n_loc = plan["n_loc"]
    out = np.empty((n_nodes, P), np.float32)
    for c in range(N_CORES):
        out[c * n_loc : (c + 1) * n_loc] = (
            res.results[c]["yt"].T[:n_loc].astype(np.float32)
        )
    return out
